# revision 2
# baseline (speedup 1.0000x reference)
"""Trainium2 Bass kernel for ExpKernelModule (Hawkes positive-likelihood intensities).

out[b,i] = sum_{j<i} alpha[u,v]*beta[u,v]*exp(clip(-beta[u,v]*(t_i-t_j), -20, 0))
with u=ct[b,i], v=ct[b,j], alpha=softplus(log_alpha), beta=softplus(log_beta).

Device algorithm (per core, one batch per core, data-parallel over B=8):
the exp argument  log(a*b) - beta*(t_i - t_j)  is a rank-64 bilinear form over
the (receiver, trigger) type one-hots:

  arg[i,j] = sum_v W[v,i]*R[v,j] + W[32+v,i]*R[32+v,j]
  W[v,i]    = C1[u_i,v] - B[u_i,v]*t_i      (C1 = log(alpha*beta), B = beta)
  W[32+v,i] = B[u_i,v]
  R[v,j]    = 1[ct_j == v]
  R[32+v,j] = t_j * 1[ct_j == v]

So per 128-row tile, one K=64 matmul (lhsT = W columns for the tile, rhs = R)
produces the full exp-argument block in PSUM; ScalarE applies Exp with a fused
accum_out row-sum. Triangularity: row tile r only needs columns [0, 128*(r+1));
the 128x128 diagonal block gets a -1e4 additive mask (VectorE) before Exp.
W and R are built on the host (O(L*D) trivial gathers), heavy O(L^2) work is
all on-device.
"""

import numpy as np

B_, L, D, P = 8, 2048, 32, 128
NT = L // P  # row tiles per batch
MASK_NEG = -1.0e4
MM_DTYPE = "float32"  # or "float32r"

_cached = {}


def _build_nc():
    import concourse.bass as bass  # noqa: F401
    import concourse.tile as tile
    from concourse import bacc, mybir

    nc = bacc.Bacc("TRN2", target_bir_lowering=False, debug=False, num_devices=8)
    w_d = nc.dram_tensor("w", (2 * D, L), mybir.dt.float32, kind="ExternalInput").ap()
    r_d = nc.dram_tensor("r", (2 * D, L), mybir.dt.float32, kind="ExternalInput").ap()
    m_d = nc.dram_tensor("m", (P, P), mybir.dt.float32, kind="ExternalInput").ap()
    o_d = nc.dram_tensor("o", (L, 1), mybir.dt.float32, kind="ExternalOutput").ap()

    mm_dt = getattr(mybir.dt, MM_DTYPE)

    with tile.TileContext(nc) as tc:
        with (
            tc.tile_pool(name="singles", bufs=1) as singles,
            tc.tile_pool(name="psum", bufs=2, space="PSUM") as psum,
            tc.tile_pool(name="acc", bufs=4) as accp,
        ):
            w_sb = singles.tile([2 * D, L], mybir.dt.float32)
            nc.sync.dma_start(w_sb[:, :], w_d[:, :])
            r_sb = singles.tile([2 * D, L], mybir.dt.float32)
            nc.sync.dma_start(r_sb[:, :], r_d[:, :])
            m_sb = singles.tile([P, P], mybir.dt.float32)
            nc.sync.dma_start(m_sb[:, :], m_d[:, :])

            for rt in range(NT):
                ncols = P * (rt + 1)
                pt = psum.tile([P, L], mybir.dt.float32)
                lhsT = w_sb[:, rt * P : (rt + 1) * P].bitcast(mm_dt)
                for c0 in range(0, ncols, 512):
                    w_len = min(512, ncols - c0)
                    nc.tensor.matmul(
                        pt[:, c0 : c0 + w_len],
                        lhsT,
                        r_sb[:, c0 : c0 + w_len].bitcast(mm_dt),
                        start=True,
                        stop=True,
                    )
                # strict-lower mask on the diagonal 128x128 block
                nc.vector.tensor_add(
                    pt[:, ncols - P : ncols], pt[:, ncols - P : ncols], m_sb[:, :]
                )
                acc = accp.tile([P, 1], mybir.dt.float32)
                nc.scalar.activation(
                    pt[:, :ncols],
                    pt[:, :ncols],
                    mybir.ActivationFunctionType.Exp,
                    accum_out=acc[:, :],
                )
                nc.sync.dma_start(o_d[rt * P : (rt + 1) * P, :], acc[:, :])

    nc.compile()
    return nc


def _softplus(x):
    return np.log1p(np.exp(-np.abs(x))) + np.maximum(x, 0.0)


def _host_prep(time_points, event_types, log_alpha, log_beta):
    t = np.asarray(time_points).astype(np.float64)  # (B, L)
    u = np.asarray(event_types).astype(np.int64)  # (B, L)
    A = _softplus(np.asarray(log_alpha).astype(np.float64))
    Bt = _softplus(np.asarray(log_beta).astype(np.float64))
    C1 = np.log(A * Bt)  # (D, D)

    Cu = C1[u]  # (B, L, D)
    Bu = Bt[u]  # (B, L, D)
    W = np.concatenate(
        [
            np.transpose(Cu - Bu * t[..., None], (0, 2, 1)),
            np.transpose(Bu, (0, 2, 1)),
        ],
        axis=1,
    ).astype(np.float32)  # (B, 2D, L)
    onehot = (u[..., None] == np.arange(D)).astype(np.float64)  # (B, L, D)
    R = np.concatenate(
        [
            np.transpose(onehot, (0, 2, 1)),
            np.transpose(onehot * t[..., None], (0, 2, 1)),
        ],
        axis=1,
    ).astype(np.float32)  # (B, 2D, L)
    mask = np.triu(np.full((P, P), MASK_NEG, dtype=np.float32), k=0)  # (P, P)
    return W, R, mask


def _run(inputs, trace=False):
    from concourse.bass_utils import run_bass_kernel_spmd

    W, R, mask = _host_prep(
        inputs["time_points"],
        inputs["event_types"],
        inputs["log_alpha"],
        inputs["log_beta"],
    )
    if "nc" not in _cached:
        _cached["nc"] = _build_nc()
    nc = _cached["nc"]

    in_maps = [{"w": W[b], "r": R[b], "m": mask} for b in range(B_)]
    bres = run_bass_kernel_spmd(
        nc, in_maps, core_ids=list(range(B_)), trace=trace,
        trace_cores=[0] if trace else None,
    )
    out = np.stack([bres.results[b]["o"].reshape(L) for b in range(B_)], axis=0)
    return out.astype(np.float32), bres


def kernel(**inputs) -> np.ndarray:
    out, _ = _run(inputs, trace=False)
    return out
